# revision 26
# baseline (speedup 1.0000x reference)
"""Dense dot-product attention on 8 Trainium2 NeuronCores.

Problem: query/key/value [32, 2048, 64] fp32 -> softmax(Q K^T / 8) V.
Sharding: batch dim split 4-per-core across 8 cores (data parallel, no
collectives). Each core computes full attention for its 4 batches.

Architecture (v2): the kernel is ACT(exp)-bound. 16.8M exps per core at
1 elem/lane/cycle @1.2GHz is a 109us floor; the design minimizes
ACTIVATE instruction overhead (222-cycle PSUM access init per
instruction) by reading score tiles from PSUM in N=2048/1024 slabs, and
keeps every other engine under the ACT's ~130us busy.

Key structural points:
  - PE does ONLY matmuls (S = QK^T row-packed strip pairs; PV with an
    extended [V | 1 | 0..] stationary so the softmax denominator and
    clean padding come along for free). ~83us/core, always under ACT.
  - ALL transposes ride the DMA xbar (dma_start_transpose with blocked
    3D outputs -- one instruction per tensor; the sync engine
    serializes DMA dispatch at ~1us/instruction, so DMA instruction
    count is kept to ~16 per batch):
      K: cast fp16, one [128,(16x64)] -> [128,8,128] blocked transpose;
         block t holds K^T of k-tile 2t in partitions 0-63 and k-tile
         2t+1 in partitions 64-127 -- exactly the row-packed strip
         layout the S matmuls want. No duplication needed.
      Q: same transpose, then 3 strided SBUF DMAs reshuffle the packed
         halves into a flat duplicated Q^T [128, 2048] (both halves
         identical).
      out: PV accumulates out^T [80, 512] in PSUM (rows 0-63 = values,
         64 = denominator, 65-79 = zeros); after a fp16 evacuation one
         [80,512] -> [128,4,80] blocked transpose yields natural-order
         output chunks; DVE reciprocal + scale produce [128, q, 64] f32.
  - PSUM budget (8 banks): squad [128,4,512] f32 = 4 banks,
    spair [128,2,512] f32 = 2 banks, pv [80,512] f32 x2 = 2 banks.
    Score slots alternate squad/spair strictly (4,2,4,2,2,2 k-tiles per
    512-wide q pass) so the exp stream never waits on a buffer.
  - Per slot the program emits S(i), exp(i), PV(i-2) (depth-2 deferral
    so S(i+1) precedes PV(i-1) in the PE queue); the deferred PV carries
    across q-pass AND batch boundaries so the PE queue never
    head-of-line blocks on an exp result and ACT never bubbles.
  - Prep is pipelined two batches deep (stage1 loads+casts two batches
    ahead, stage2 transposes+reshuffle one ahead) because each DMA hop
    costs 2-5us completion latency on the serial sync dispatch queue.
"""

import numpy as np

B, L, D = 32, 2048, 64
NCORES = 8
B_SH = B // NCORES          # 4 batches per core
LT = L // 128               # 16 k-tiles of 128
NBLK = LT // 2              # 8 packed k-tile-pair blocks
NQH = 4                     # q processed in 512-wide passes
QHW = L // NQH              # 512
DP = 80                     # padded out^T partitions (64 vals + denom + 15 zeros)
SCALE = 1.0 / np.sqrt(np.float32(D))  # 0.125

# slot pattern per q pass: (pool_name, packed blocks) -- sizes 4,2,4,2,2,2
# k-tiles; strict squad/spair alternation keeps exp back-to-back.
SLOTS = [("sq", (0, 1)), ("sp", (2,)), ("sq", (3, 4)),
         ("sp", (5,)), ("sq", (6,)), ("sp", (7,))]

_cached = {}


def _build():
    import concourse.bacc as bacc
    import concourse.tile as tile
    from concourse import mybir

    f32 = mybir.dt.float32
    fp16 = mybir.dt.float16
    Exp = mybir.ActivationFunctionType.Exp

    nc = bacc.Bacc("TRN2", target_bir_lowering=False, debug=False)

    q_d = nc.dram_tensor("query", [B_SH, L, D], f32, kind="ExternalInput")
    k_d = nc.dram_tensor("key", [B_SH, L, D], f32, kind="ExternalInput")
    v_d = nc.dram_tensor("value", [B_SH, L, D], f32, kind="ExternalInput")
    o_d = nc.dram_tensor("out", [B_SH, L, D], f32, kind="ExternalOutput")

    with tile.TileContext(nc) as tc:
        with (
            tc.tile_pool(name="consts", bufs=1) as consts,
            tc.tile_pool(name="nat", bufs=6) as natp,
            tc.tile_pool(name="h16", bufs=4) as h16p,
            tc.tile_pool(name="pk", bufs=4) as pkp,
            tc.tile_pool(name="qt", bufs=3) as qtp,
            tc.tile_pool(name="vr", bufs=3) as vrp,
            tc.tile_pool(name="er", bufs=5) as erp,
            tc.tile_pool(name="pvsb", bufs=3) as pvsbp,
            tc.tile_pool(name="ot", bufs=3) as otp,
            tc.tile_pool(name="rz", bufs=6) as rzp,
            tc.tile_pool(name="oall", bufs=2) as oallp,
            tc.tile_pool(name="sq", bufs=1, space="PSUM") as sqp,
            tc.tile_pool(name="sp", bufs=1, space="PSUM") as spp,
            tc.tile_pool(name="pv", bufs=2, space="PSUM") as pvp,
        ):
            pools = {"sq": sqp, "sp": spp}

            # ACT exp table load primed first; PE warmers keep HAM busy
            # through batch-0 prep (DMA/DVE only, no PE work in prep).
            wsrc = consts.tile([128, 512], fp16)
            nc.vector.memset(wsrc, 1.0)
            dummy = consts.tile([128, 1], f32)
            nc.vector.memset(dummy, 0.0)
            nc.scalar.activation(out=dummy, in_=dummy, func=Exp, scale=1.0)
            # identity (gpsimd, idle at boot) for the batch-0 PE-transpose
            # fast path -- 2 fewer DMA hops than the xbar+reshuffle chain
            identh = consts.tile([128, 128], fp16)
            from concourse.masks import make_identity
            make_identity(nc, identh)

            warm_flip = [0]

            def warmer():
                # use the score pools' own "s" tag: no extra PSUM footprint
                pool = (sqp, spp)[warm_flip[0] % 2]
                warm_flip[0] += 1
                wt = pool.tile([64, 512], f32, tag="s", name="warm")
                nc.tensor.matmul(wt, wsrc[:, 0:64], wsrc,
                                 start=True, stop=True, skip_group_check=True)

            # ---------------- prep: loads, casts, xbar transposes -----------
            qkT = {}   # b -> (qhT [128,2048] fp16 dup-halves, k_pk [128,8,128])
            v_r = {}   # b -> vr [128, LT, DP] fp16

            def prep_jobs(b, split_fine=False):
                """Jobs (DMA/DVE only, no PE) preparing batch b's operands.
                Returns (head, rest). All DMA instructions are coarse: the
                sync engine serializes dispatch at ~1us per instruction."""
                q_nat = natp.tile([128, LT, D], f32, tag="qnat")
                k_nat = natp.tile([128, LT, D], f32, tag="knat")
                v_nat = natp.tile([128, LT, D], f32, tag="vnat")
                qh_nat = h16p.tile([128, LT, D], fp16, tag="qh")
                kh_nat = h16p.tile([128, LT, D], fp16, tag="kh")
                q_pk = pkp.tile([128, NBLK, 128], fp16, tag="qpk")
                k_pk = pkp.tile([128, NBLK, 128], fp16, tag="kpk")
                qhT = qtp.tile([128, L], fp16, tag="qhT")
                vr = vrp.tile([128, LT, DP], fp16, tag="vr")
                qkT[b] = (qhT, k_pk)
                v_r[b] = vr

                q_r = q_d.ap()[b].rearrange("(t p) d -> p t d", p=128)
                k_r = k_d.ap()[b].rearrange("(t p) d -> p t d", p=128)
                v_rr = v_d.ap()[b].rearrange("(t p) d -> p t d", p=128)

                def load(dst, src, lo, hi):
                    def job():
                        nc.sync.dma_start(out=dst[:, lo:hi, :],
                                          in_=src[:, lo:hi, :])
                    return job

                def cast(dst, src, lo, hi):
                    def job():
                        nc.vector.tensor_copy(out=dst[:, lo:hi, :],
                                              in_=src[:, lo:hi, :])
                    return job

                def xbar(dst, src, blo, bhi, eng=None):
                    # blocked transpose: src k-tiles [2*blo, 2*bhi) ->
                    # dst[:, blo:bhi, :]; one DMA instruction.
                    def job():
                        (eng or nc.sync).dma_start_transpose(
                            out=dst[:, blo:bhi, :],
                            in_=src[:, 2 * blo:2 * bhi, :])
                    return job

                def reshuffle(glo, ghi, eng=None):
                    # q-groups [glo, ghi): q-tiles 4g..4g+3 <- packed blocks
                    # 2g, 2g+1. Even tiles from q_pk[0:64], odd from
                    # q_pk[64:128]; both qhT halves written directly from
                    # q_pk (4 parallel DMAs, no serial duplication hop).
                    def job():
                        for half in (slice(0, 64), slice(64, 128)):
                            span = qhT[half, glo * 512:ghi * 512].rearrange(
                                "p (t e c) -> p t e c", e=2, c=128)
                            (eng or nc.sync).dma_start(
                                out=span[:, :, 0, :],
                                in_=q_pk[0:64, 2 * glo:2 * ghi, :])
                            (eng or nc.sync).dma_start(
                                out=span[:, :, 1, :],
                                in_=q_pk[64:128, 2 * glo:2 * ghi, :])
                    return job

                def vfill(lo, hi):
                    def job():
                        nc.vector.tensor_copy(out=vr[:, lo:hi, 0:D],
                                              in_=v_nat[:, lo:hi, :])
                        nc.vector.memset(vr[:, lo:hi, D:D + 1], 1.0)
                        nc.vector.memset(vr[:, lo:hi, D + 1:DP], 0.0)
                    return job

                if split_fine:
                    # batch 0: stage the first slot's operands first; the
                    # rest is ordered by consumption deadline and must be
                    # ISSUED before the slot that consumes it (trace order =
                    # dep order); the caller's weave schedule covers it.
                    def pe_k(blk):
                        # one PE pair-transpose lands the packed
                        # [tile 2b; tile 2b+1] halves of a K block directly.
                        tp = pvp.tile([128, 128], fp16, tag="pv", name="ktr")
                        nc.tensor.transpose(
                            tp, kh_nat[:, 2 * blk:2 * blk + 2, :], identh)
                        nc.vector.tensor_copy(out=k_pk[:, blk, :], in_=tp)

                    def pe_q(t):
                        # transpose twice (col positions 0 and 64) so both
                        # duplicated qhT halves come straight from PSUM with
                        # one DVE copy, no reshuffle DMAs.
                        tp = pvp.tile([128, 128], fp16, tag="pv", name="qtr")
                        nc.tensor.transpose(tp[0:64, :], qh_nat[:, t, :],
                                            identh)
                        nc.tensor.transpose(tp[64:128, :], qh_nat[:, t, :],
                                            identh)
                        nc.vector.tensor_copy(
                            out=qhT[:, t * 128:(t + 1) * 128], in_=tp)

                    def fast_a():
                        # minimal slot-0 operands: K blocks 0-1, Q tiles 0-3
                        for blk in (0, 1):
                            pe_k(blk)
                        for t in range(4):
                            pe_q(t)

                    def fast_b():
                        # rest of qh0's K blocks + qh1's Q tiles; woven at
                        # slot 0 so all tp tiles still rotate the pv pool
                        # BEFORE any pv accumulator goes live.
                        for blk in (2, 3, 4):
                            pe_k(blk)
                        for t in range(4, 8):
                            pe_q(t)

                    head = [
                        load(q_nat, q_r, 0, 4), load(k_nat, k_r, 0, 4),
                        cast(qh_nat, q_nat, 0, 4), cast(kh_nat, k_nat, 0, 4),
                        fast_a,
                        load(k_nat, k_r, 4, 10), load(q_nat, q_r, 4, 8),
                        load(v_nat, v_rr, 0, 4), vfill(0, 4),
                    ]
                    rest = [
                        cast(kh_nat, k_nat, 4, 10), cast(qh_nat, q_nat, 4, 8),
                        fast_b,
                        load(k_nat, k_r, 10, LT), cast(kh_nat, k_nat, 10, LT),
                        xbar(k_pk, kh_nat, 5, NBLK),
                        load(v_nat, v_rr, 4, 8), vfill(4, 8),
                        load(q_nat, q_r, 8, LT), cast(qh_nat, q_nat, 8, LT),
                        xbar(q_pk, qh_nat, 4, NBLK), reshuffle(2, 4),
                        load(v_nat, v_rr, 8, LT), vfill(8, LT),
                    ]
                    return head, rest
                # steady-state: stage1 (loads + casts) is woven TWO batches
                # ahead; stage2 (xbar transposes + reshuffle + vr build) one
                # batch ahead -- the multi-hop DMA chain then has a full
                # batch (~30us) of slack before its consumers.
                stage1 = [
                    load(k_nat, k_r, 0, LT), cast(kh_nat, k_nat, 0, LT),
                    load(q_nat, q_r, 0, LT), cast(qh_nat, q_nat, 0, LT),
                    load(v_nat, v_rr, 0, LT),
                ]
                stage2 = [
                    xbar(k_pk, kh_nat, 0, NBLK),
                    xbar(q_pk, qh_nat, 0, NBLK), reshuffle(0, 4),
                    vfill(0, LT),
                ]
                return stage1, stage2

            # ---------------- main: S -> exp -> PV stream -------------------
            pending = []       # deferred DVE/DMA jobs woven into the stream
            deferred = []      # PV emission delayed TWO slots (so S(i+1)
                               # precedes PV(i-1) in the PE queue and the PE
                               # never head-of-line blocks on an exp result);
                               # crosses q-pass and batch boundaries.
            pv_cur = [None]

            def out_jobs(b, qh, pv):
                """Output path for one q pass: evac, blocked xbar T, scale,
                store. Kept per-q-pass so only the last pass's path is
                kernel-tail latency; the very last pass transposes on the PE
                (PSUM is free by then) to skip the slow xbar-DMA hop."""
                pv_sb = pvsbp.tile([DP, QHW], fp16, tag="pvsb")
                ot = otp.tile([128, QHW // 128, DP], fp16, tag="ot")
                o_all = oallp.tile([128, QHW // 128, D], f32, tag="oall")
                last = (b == B_SH - 1 and qh == NQH - 1)

                def evac():
                    nc.vector.tensor_copy(out=pv_sb, in_=pv)

                def trans():
                    nc.sync.dma_start_transpose(out=ot, in_=pv_sb)

                jobs = [evac] if last else [evac, trans]
                for qt in range(QHW // 128):
                    if last:
                        def norm(qt=qt):
                            tp = sqp.tile([128, DP], fp16, tag="s",
                                          name="otr")
                            nc.tensor.transpose(
                                tp, pv_sb[:, qt * 128:(qt + 1) * 128],
                                identh[0:DP, 0:DP])
                            rz = rzp.tile([128, 1], f32, tag="rz")
                            nc.vector.reciprocal(out=rz, in_=tp[:, D:D + 1])
                            nc.vector.tensor_scalar_mul(
                                out=o_all[:, qt, :], in0=tp[:, 0:D],
                                scalar1=rz)
                    else:
                        def norm(qt=qt):
                            rz = rzp.tile([128, 1], f32, tag="rz")
                            nc.vector.reciprocal(out=rz,
                                                 in_=ot[:, qt, D:D + 1])
                            nc.vector.tensor_scalar_mul(
                                out=o_all[:, qt, :], in0=ot[:, qt, 0:D],
                                scalar1=rz)
                    jobs.append(norm)

                def store():
                    nc.sync.dma_start(
                        out=o_d.ap()[b, qh * QHW:(qh + 1) * QHW, :].rearrange(
                            "(t p) d -> p t d", p=128),
                        in_=o_all)
                jobs.append(store)
                return jobs

            def emit_one_deferred():
                db, dqh, de, dkts, dvr = deferred.pop(0)
                if dkts[0] == 0:
                    pv_cur[0] = pvp.tile([DP, QHW], f32, tag="pv", name="pv")
                dpv = pv_cur[0]
                for j, kt in enumerate(dkts):
                    nc.tensor.matmul(dpv, dvr[:, kt, :], de[:, j, :],
                                     start=(kt == 0), stop=(kt == LT - 1))
                if dkts[-1] == LT - 1:
                    pending.extend(out_jobs(db, dqh, dpv))

            def main(b, next_jobs, weave=1, weave0=None):
                # weave0: jobs/slot for the first 3 slots (lets batch-0 front
                # its own prep without flooding the serial sync queue with
                # later batches' loads)
                qhT, k_pk = qkT.pop(b)
                vr = v_r.pop(b)
                nslot = 0
                slot_no = [0]
                for qh in range(NQH):
                    qs = slice(qh * QHW, (qh + 1) * QHW)
                    for pool_name, blocks in SLOTS:
                        pool = pools[pool_name]
                        nkt = 2 * len(blocks)
                        s = pool.tile([128, nkt, QHW], f32, tag="s")
                        kts = []
                        for i, blk in enumerate(blocks):
                            nc.tensor.matmul(
                                s[:, 2 * i, :], k_pk[0:64, blk, :],
                                qhT[0:64, qs], start=True, stop=True)
                            nc.tensor.matmul(
                                s[:, 2 * i + 1, :], k_pk[64:128, blk, :],
                                qhT[64:128, qs], start=True, stop=True)
                            kts += [2 * blk, 2 * blk + 1]
                        e = erp.tile([128, nkt, QHW], fp16, tag="e")
                        nc.scalar.activation(out=e, in_=s, func=Exp,
                                             scale=float(SCALE))
                        deferred.append((b, qh, e, kts, vr))
                        if len(deferred) > 2:
                            emit_one_deferred()
                        # weave deferred output jobs + next-batch prep
                        for _ in range(2):
                            if pending:
                                pending.pop(0)()
                        if weave0 is not None and slot_no[0] < len(weave0):
                            w = weave0[slot_no[0]]
                        else:
                            w = weave
                        slot_no[0] += 1
                        for _ in range(w):
                            if nslot < len(next_jobs):
                                next_jobs[nslot]()
                                nslot += 1
                while nslot < len(next_jobs):
                    next_jobs[nslot]()
                    nslot += 1

            for _ in range(10):
                warmer()
            head0, rest0 = prep_jobs(0, split_fine=True)
            for job in head0:
                job()
            stages = {}
            for bb in range(1, B_SH):
                stages[bb] = prep_jobs(bb)
            for b in range(B_SH):
                nxt = []
                if b == 0:
                    s1_1, s2_1 = stages[1]
                    s1_2, _ = stages[2]
                    nxt = rest0 + s1_1 + s2_1 + s1_2
                else:
                    if b + 1 < B_SH:
                        nxt += stages[b + 1][1]     # stage2(b+1)
                    if b + 2 < B_SH:
                        nxt += stages[b + 2][0]     # stage1(b+2)
                if b == 0:
                    main(b, nxt, weave=2, weave0=[4, 4, 4, 4, 4])
                else:
                    main(b, nxt, weave=2)
            while deferred:
                emit_one_deferred()
            for job in pending:
                job()

    nc.finalize()
    return nc


def _get_nc():
    if "nc" not in _cached:
        _cached["nc"] = _build()
    return _cached["nc"]


def kernel(query, key, value):
    from concourse.bass_utils import run_bass_kernel_spmd

    nc = _get_nc()
    query = np.ascontiguousarray(query, dtype=np.float32)
    key = np.ascontiguousarray(key, dtype=np.float32)
    value = np.ascontiguousarray(value, dtype=np.float32)

    in_maps = []
    for c in range(NCORES):
        sl = slice(c * B_SH, (c + 1) * B_SH)
        in_maps.append({
            "query": query[sl], "key": key[sl], "value": value[sl]})

    res = run_bass_kernel_spmd(nc, in_maps, core_ids=list(range(NCORES)))
    out = np.concatenate([r["out"] for r in res.results], axis=0)
    return out
